# revision 1
# baseline (speedup 1.0000x reference)
"""Trainium2 Bass kernel for nn_BaselineTargetHead (per-sample dynamic MLP).

Strategy: data-parallel over 8 NeuronCores, 8 samples per core.
Per sample the chain is 5 per-sample linear layers over 64 spatial positions:
  [1024,2048] @ [2048,64] -> sigmoid -> ... -> [1,128] @ [128,64] + b

Device kernel (per core, per sample):
  - weights arrive as pre-transposed fp16 "slabs" laid out exactly as the
    SBUF image [128 part, sum_l (Cin_l/128)*Cout_l cols] so a single large
    contiguous DMA loads a sample's full weight set (double-buffered).
  - matmul: lhsT = W^T tile [128(Cin), 128(Cout)], rhs = activation tile
    [128(Cin), 64(spatial)], accumulate over Cin tiles in PSUM fp32.
  - ScalarE applies bias+sigmoid fused, writing fp16 activation tiles that
    feed the next layer without any transposition.
"""

import numpy as np

import concourse.bass as bass
import concourse.mybir as mybir
import concourse.tile as tile
from concourse.bass_utils import run_bass_kernel_spmd

N_CORES = 8
B = 64
S_PER_CORE = B // N_CORES  # 8 samples per core
HW = 64  # 8x8 spatial positions
DIMS = [2048, 1024, 512, 256, 128, 1]
LAYERS = [(2048, 1024), (1024, 512), (512, 256), (256, 128)]  # (Cin, Cout) of fc1..fc4
W_COLS = sum((ci // 128) * co for ci, co in LAYERS)  # 21760 fp16 cols per sample
X_COLS = (2048 // 128) * HW  # 1024
XW5_COLS = X_COLS + 32  # w5 in col X_COLS, zero-padded to 32 cols for a legal M=32 matmul
# bias image columns per sample: fc1 m0..7 | fc2 m0..3 | fc3 m0..1 | fc4 m0 | fc5
BIAS_COL0 = [0, 8, 12, 14]
BIAS_COLS = 16

def _split_ctrl_multiwaits(nc):
    """walrus in this env rejects >1 sync-wait per instruction. Move extra
    waits onto NOPs placed immediately before, on the same engine — engines
    execute in order, so this is semantically identical."""
    n_fixed = 0
    for bb in nc.main_func.blocks:
        insts = bb.instructions
        i = 0
        while i < len(insts):
            ins = insts[i]
            si = ins.sync_info
            if si is not None and si.on_wait and len(si.on_wait) > 1:
                waits = list(si.on_wait)
                new_nops = []
                for j, w in enumerate(waits[1:]):
                    nop = mybir.InstNoOp(name=f"{ins.name}-splitw-{j}", ins=[], outs=[])
                    nop.engine = ins.engine
                    nop.sync_info = mybir.SyncInfo(on_update=[], on_wait=[w])
                    new_nops.append(nop)
                si.on_wait = [waits[0]]
                insts[i:i] = new_nops
                i += len(new_nops)
                n_fixed += 1
            i += 1
    return n_fixed


def _build_nc():
    f16 = mybir.dt.float16
    f32 = mybir.dt.float32
    nc = bass.Bass()
    wslab_d = nc.dram_tensor("wslab", [S_PER_CORE, 128, W_COLS], f16, kind="ExternalInput")
    xw5_d = nc.dram_tensor("xw5", [128, S_PER_CORE * XW5_COLS], f16, kind="ExternalInput")
    bias_d = nc.dram_tensor("bias", [128, S_PER_CORE * BIAS_COLS], f32, kind="ExternalInput")
    out_d = nc.dram_tensor("out", [S_PER_CORE, HW], f32, kind="ExternalOutput")

    sig = mybir.ActivationFunctionType.Sigmoid
    ident = mybir.ActivationFunctionType.Identity

    # L1 weights in slab part A, L2-L4 in part B (separate tiles let layer-1
    # matmuls start before the whole slab has landed)
    A_COLS = (LAYERS[0][0] // 128) * LAYERS[0][1]  # 16384
    B_COLS = W_COLS - A_COLS  # 5376

    with tile.TileContext(nc) as tc:
        with (
            tc.tile_pool(name="wpool", bufs=3) as wpool,
            tc.tile_pool(name="qpool", bufs=2) as qpool,
            tc.tile_pool(name="misc", bufs=1) as misc,
            tc.tile_pool(name="psum", bufs=6, space="PSUM") as psum_pool,
        ):
            # small inputs: one DMA each, issued on the ACT HWDGE queue so the
            # SP queue carries nothing but the big weight-slab stream
            bias_sb = misc.tile([128, S_PER_CORE * BIAS_COLS], f32)
            nc.scalar.dma_start(bias_sb[:], bias_d[:])
            xw5_sb = misc.tile([128, S_PER_CORE * XW5_COLS], f16)
            nc.scalar.dma_start(xw5_sb[:], xw5_d[:])

            HA = A_COLS // 2
            for s in range(S_PER_CORE):
                wta1 = wpool.tile([128, HA], f16, tag="wslabA1")
                nc.sync.dma_start(wta1[:], wslab_d[s, :, 0:HA])
                wta2 = wpool.tile([128, HA], f16, tag="wslabA2")
                nc.sync.dma_start(wta2[:], wslab_d[s, :, HA:A_COLS])
                wtb = wpool.tile([128, B_COLS], f16, tag="wslabB")
                nc.sync.dma_start(wtb[:], wslab_d[s, :, A_COLS:W_COLS])

                xt = xw5_sb[:, s * XW5_COLS : (s + 1) * XW5_COLS]
                q_prev = xt[:, 0:X_COLS]
                # per-layer column offset within its slab tile (A holds L1,
                # B holds L2..L4 back to back)
                layer_off = [0, 0]
                for cin, cout in LAYERS[1:-1]:
                    layer_off.append(layer_off[-1] + (cin // 128) * cout)
                for li, (cin, cout) in enumerate(LAYERS):
                    kt, mt = cin // 128, cout // 128
                    off = layer_off[li]
                    qn = qpool.tile([128, mt * HW], f16, tag=f"q{li}")
                    for m in range(mt):
                        ps = psum_pool.tile([128, HW], f32, tag="ps")
                        for k in range(kt):
                            if li == 0:
                                col = k * cout + m * 128
                                wt, wcol = (wta1, col) if col < HA else (wta2, col - HA)
                            else:
                                wt, wcol = wtb, off + k * cout + m * 128
                            lhsT = wt[:, wcol : wcol + 128]
                            rhs = q_prev[:, k * HW : (k + 1) * HW]
                            nc.tensor.matmul(
                                ps[:], lhsT, rhs, start=(k == 0), stop=(k == kt - 1)
                            )
                        bcol = s * BIAS_COLS + BIAS_COL0[li] + m
                        nc.scalar.activation(
                            qn[:, m * HW : (m + 1) * HW],
                            ps[:],
                            sig,
                            bias=bias_sb[:, bcol : bcol + 1],
                            scale=1.0,
                        )
                    q_prev = qn[:]

                ps5 = psum_pool.tile([128, HW], f32, tag="ps", name="ps5")
                nc.tensor.matmul(
                    ps5[0:32, :], xt[:, X_COLS:XW5_COLS], q_prev[:, 0:HW], start=True, stop=True
                )
                b5col = s * BIAS_COLS + 15
                ot5 = qpool.tile([128, HW], f32, tag="ot5", name="ot5")
                nc.scalar.activation(
                    ot5[:], ps5[:], ident, bias=bias_sb[:, b5col : b5col + 1], scale=1.0
                )
                nc.scalar.dma_start(out_d[s : s + 1, :], ot5[0:1, :])

    _split_ctrl_multiwaits(nc)
    return nc


_NC_CACHE = None


def _get_nc():
    global _NC_CACHE
    if _NC_CACHE is None:
        _NC_CACHE = _build_nc()
    return _NC_CACHE


def _prep_core(inputs, c):
    """Build the per-core input map (numpy only, host-side layout prep)."""
    sl = slice(c * S_PER_CORE, (c + 1) * S_PER_CORE)

    wparts = []
    for li, (cin, cout) in enumerate(LAYERS):
        w = inputs[f"target_fc{li + 1}w"][sl, :, :, 0, 0]  # [S, Cout, Cin]
        # -> [S, 128, (Cin/128)*Cout] with img[s, p, k*Cout+co] = w[s, co, k*128+p]
        wt = w.transpose(0, 2, 1).reshape(S_PER_CORE, cin // 128, 128, cout)
        wt = wt.transpose(0, 2, 1, 3).reshape(S_PER_CORE, 128, -1)
        wparts.append(wt.astype(np.float16))
    wslab = np.ascontiguousarray(np.concatenate(wparts, axis=2))

    x = inputs["target_in_vec"][sl].reshape(S_PER_CORE, 2048 // 128, 128, HW)
    ximg = x.transpose(0, 2, 1, 3).reshape(S_PER_CORE, 128, X_COLS).astype(np.float16)
    w5 = inputs["target_fc5w"][sl, 0, :, 0, 0].astype(np.float16)  # [S, 128]
    w5pad = np.zeros((S_PER_CORE, 128, 32), np.float16)
    w5pad[:, :, 0] = w5
    # partition-major [128, S*XW5_COLS] so the DMA is one big 2D copy
    xw5 = np.ascontiguousarray(
        np.concatenate([ximg, w5pad], axis=2).transpose(1, 0, 2).reshape(128, -1)
    )

    bias = np.zeros((S_PER_CORE, 128, BIAS_COLS), np.float32)
    for li, (cin, cout) in enumerate(LAYERS):
        b = inputs[f"target_fc{li + 1}b"][sl]  # [S, Cout]
        bias[:, :, BIAS_COL0[li] : BIAS_COL0[li] + cout // 128] = b.reshape(
            S_PER_CORE, cout // 128, 128
        ).transpose(0, 2, 1)
    bias[:, 0, 15] = inputs["target_fc5b"][sl, 0]
    bias = np.ascontiguousarray(bias.transpose(1, 0, 2).reshape(128, -1))

    return {"wslab": wslab, "xw5": xw5, "bias": bias}


def kernel(**inputs):
    inputs = {k: np.asarray(v) for k, v in inputs.items()}
    nc = _get_nc()
    in_maps = [_prep_core(inputs, c) for c in range(N_CORES)]
    res = run_bass_kernel_spmd(nc, in_maps, list(range(N_CORES)))
    out = np.concatenate([np.asarray(res.results[c]["out"]) for c in range(N_CORES)], axis=0)
    return out.reshape(B, 8, 8).astype(np.float32)



# revision 9
# speedup vs baseline: 1.6064x; 1.6064x over previous
"""Trainium2 Bass kernel for nn_BaselineTargetHead (per-sample dynamic MLP).

Strategy: data-parallel over 8 NeuronCores, 8 samples per core.
Per sample the chain is 5 per-sample linear layers over 64 spatial positions:
  [1024,2048] @ [2048,64] -> sigmoid -> ... -> [1,128] @ [128,64] + b

The kernel is HBM-bound (weights are used exactly once), so weights and the
layer-1 input travel as fp8 e3m4 (4 mantissa bits). Weights are pre-scaled by
64 on the host so they sit in e3m4's normal range; the 1/64 is folded into the
ScalarE activation's free affine (out = sigmoid(scale*psum + bias)).
Activations stay fp16, so layers 2-5 run mixed fp8-lhsT x fp16-rhs matmuls
(legal: only fp32 must be paired with fp32).

Device kernel (per core, per sample):
  - one packed per-sample fp8 slab [x | w5 | L1 | L2 | L3 | L4], with each
    layer's weights m-block-major so DMA chunk order == compute order. Four
    DMA chunks per sample (x+w5, L1 m0-3, L1 m4-7, L2-L4) on the sync ring,
    4-deep buffering so the ring never waits on buffer recycling.
  - matmul: lhsT = W^T tile [128(Cin), 128(Cout)] fp8, rhs = activation tile
    [128(Cin), 64(spatial)], accumulate over Cin tiles in PSUM fp32.
  - ScalarE applies scale+bias+sigmoid fused, writing fp16 activation tiles
    that feed the next layer without any transposition.
  - per-sample [1,64] results collect into one SBUF tile; single output DMA.
"""

import numpy as np
import ml_dtypes

import concourse.bass as bass
import concourse.mybir as mybir
import concourse.tile as tile
from concourse.bass_utils import run_bass_kernel_spmd

N_CORES = 8
B = 64
S_PER_CORE = B // N_CORES  # 8 samples per core
HW = 64  # 8x8 spatial positions
DIMS = [2048, 1024, 512, 256, 128, 1]
LAYERS = [(2048, 1024), (1024, 512), (512, 256), (256, 128)]  # (Cin, Cout) of fc1..fc4
W_SCALE_FP8 = 64.0  # lift weights into e3m4's normal range; undone in the act scale
FP8_CLIP = 15.0  # e3m4 saturates to inf above 15.5

X_COLS = (2048 // 128) * HW  # 1024
W5_COLS = 32  # w5 in col 0, zero-padded to 32 cols for a legal M=32 matmul
L_COLS = [(ci // 128) * co for ci, co in LAYERS]  # 16384, 4096, 1024, 256
# slab column map: [x | L1a (m0-3) | L1b (m4-7) | L2 | L3 | L4]
C0_END = X_COLS  # 1024
C1_END = C0_END + L_COLS[0] // 2  # 9216
C2_END = C1_END + L_COLS[0] // 2  # 17408
TOT_COLS = C2_END + L_COLS[1] + L_COLS[2] + L_COLS[3]  # 22784
L3_OFF = L_COLS[1]  # offset of L3 inside the C3 chunk
L4_OFF = L_COLS[1] + L_COLS[2]
# bias image columns per sample: fc1 m0..7 | fc2 m0..3 | fc3 m0..1 | fc4 m0 | fc5
BIAS_COL0 = [0, 8, 12, 14]
BIAS_COLS = 16

def _split_ctrl_multiwaits(nc):
    """walrus in this env rejects >1 sync-wait per instruction. Move extra
    waits onto NOPs placed immediately before, on the same engine — engines
    execute in order, so this is semantically identical."""
    n_fixed = 0
    for bb in nc.main_func.blocks:
        insts = bb.instructions
        i = 0
        while i < len(insts):
            ins = insts[i]
            si = ins.sync_info
            if si is not None and si.on_wait and len(si.on_wait) > 1:
                waits = list(si.on_wait)
                new_nops = []
                for j, w in enumerate(waits[1:]):
                    nop = mybir.InstNoOp(name=f"{ins.name}-splitw-{j}", ins=[], outs=[])
                    nop.engine = ins.engine
                    nop.sync_info = mybir.SyncInfo(on_update=[], on_wait=[w])
                    new_nops.append(nop)
                si.on_wait = [waits[0]]
                insts[i:i] = new_nops
                i += len(new_nops)
                n_fixed += 1
            i += 1
    return n_fixed


def _build_nc():
    f8 = mybir.dt.float8e3
    f16 = mybir.dt.float16
    f32 = mybir.dt.float32
    nc = bass.Bass()
    slab_d = nc.dram_tensor("slab", [S_PER_CORE, 128, TOT_COLS], f8, kind="ExternalInput")
    # final-layer weights stay fp16: their quantization error hits the output
    # with no sigmoid attenuation (fp8 w5 alone costs ~2% rel err)
    w5_d = nc.dram_tensor("w5", [128, S_PER_CORE * W5_COLS], f16, kind="ExternalInput")
    bias_d = nc.dram_tensor("bias", [128, S_PER_CORE * BIAS_COLS], f32, kind="ExternalInput")
    out_d = nc.dram_tensor("out", [S_PER_CORE, HW], f32, kind="ExternalOutput")

    sig = mybir.ActivationFunctionType.Sigmoid
    ident = mybir.ActivationFunctionType.Identity
    inv_s = 1.0 / W_SCALE_FP8

    with tile.TileContext(nc) as tc:
        with (
            tc.tile_pool(name="wpool", bufs=4) as wpool,
            tc.tile_pool(name="qpool", bufs=2) as qpool,
            tc.tile_pool(name="misc", bufs=1) as misc,
            tc.tile_pool(name="psum", bufs=6, space="PSUM") as psum_pool,
        ):
            # small inputs on the ACT HWDGE ring so the SP ring carries
            # nothing but the per-sample slab stream
            bias_sb = misc.tile([128, S_PER_CORE * BIAS_COLS], f32)
            nc.scalar.dma_start(bias_sb[:], bias_d[:])
            w5_sb = misc.tile([128, S_PER_CORE * W5_COLS], f16)
            nc.scalar.dma_start(w5_sb[:], w5_d[:])
            collect = misc.tile([1, S_PER_CORE * HW], f32)

            for s in range(S_PER_CORE):
                c0 = wpool.tile([128, C0_END], f8, tag="c0")
                nc.sync.dma_start(c0[:], slab_d[s, :, 0:C0_END])
                c1 = wpool.tile([128, C1_END - C0_END], f8, tag="c1")
                nc.sync.dma_start(c1[:], slab_d[s, :, C0_END:C1_END])
                c2 = wpool.tile([128, C2_END - C1_END], f8, tag="c2")
                nc.sync.dma_start(c2[:], slab_d[s, :, C1_END:C2_END])
                c3 = wpool.tile([128, TOT_COLS - C2_END], f8, tag="c3")
                nc.sync.dma_start(c3[:], slab_d[s, :, C2_END:TOT_COLS])

                q_prev = c0[:, 0:X_COLS]
                for li, (cin, cout) in enumerate(LAYERS):
                    kt, mt = cin // 128, cout // 128
                    qn = qpool.tile([128, mt * HW], f16, tag=f"q{li}")
                    for m in range(mt):
                        ps = psum_pool.tile([128, HW], f32, tag="ps")
                        for k in range(kt):
                            if li == 0:
                                wt, wcol = (c1, (m * kt + k) * 128) if m < 4 else (
                                    c2, ((m - 4) * kt + k) * 128)
                            elif li == 1:
                                wt, wcol = c3, (m * kt + k) * 128
                            elif li == 2:
                                wt, wcol = c3, L3_OFF + (m * kt + k) * 128
                            else:
                                wt, wcol = c3, L4_OFF + k * 128
                            lhsT = wt[:, wcol : wcol + 128]
                            rhs = q_prev[:, k * HW : (k + 1) * HW]
                            nc.tensor.matmul(
                                ps[:], lhsT, rhs, start=(k == 0), stop=(k == kt - 1)
                            )
                        bcol = s * BIAS_COLS + BIAS_COL0[li] + m
                        nc.scalar.activation(
                            qn[:, m * HW : (m + 1) * HW],
                            ps[:],
                            sig,
                            bias=bias_sb[:, bcol : bcol + 1],
                            scale=inv_s,
                        )
                    q_prev = qn[:]

                ps5 = psum_pool.tile([128, HW], f32, tag="ps", name="ps5")
                nc.tensor.matmul(
                    ps5[0:32, :], w5_sb[:, s * W5_COLS : (s + 1) * W5_COLS],
                    q_prev[:, 0:HW], start=True, stop=True,
                )
                b5col = s * BIAS_COLS + 15
                nc.scalar.activation(
                    collect[0:1, s * HW : (s + 1) * HW],
                    ps5[0:1, :],
                    ident,
                    bias=bias_sb[0:1, b5col : b5col + 1],
                    scale=1.0,
                )
            nc.scalar.dma_start(out_d[:], collect[:])

    _split_ctrl_multiwaits(nc)
    return nc


_NC_CACHE = None


def _get_nc():
    global _NC_CACHE
    if _NC_CACHE is None:
        _NC_CACHE = _build_nc()
    return _NC_CACHE


def _to_fp8(a):
    return np.clip(a, -FP8_CLIP, FP8_CLIP).astype(ml_dtypes.float8_e3m4)


def _prep_core(inputs, c):
    """Build the per-core input map (numpy only, host-side layout prep)."""
    sl = slice(c * S_PER_CORE, (c + 1) * S_PER_CORE)

    # x image: [S, 128, 1024] with img[s, p, k*64+h] = x[s, k*128+p, h]
    x = inputs["target_in_vec"][sl].reshape(S_PER_CORE, 2048 // 128, 128, HW)
    ximg = _to_fp8(x.transpose(0, 2, 1, 3).reshape(S_PER_CORE, 128, X_COLS))
    w5pad = np.zeros((S_PER_CORE, 128, W5_COLS), np.float16)
    w5pad[:, :, 0] = inputs["target_fc5w"][sl, 0, :, 0, 0]  # [S, 128]
    w5img = np.ascontiguousarray(
        w5pad.transpose(1, 0, 2).reshape(128, S_PER_CORE * W5_COLS)
    )

    # per-layer m-block-major weight images:
    # img[s, p, (m*kt+k)*128 + c] = w[s, m*128+c, k*128+p] * 64
    wparts = []
    for li, (cin, cout) in enumerate(LAYERS):
        kt, mt = cin // 128, cout // 128
        w = inputs[f"target_fc{li + 1}w"][sl, :, :, 0, 0]  # [S, Cout, Cin]
        wt = w.reshape(S_PER_CORE, mt, 128, kt, 128)  # [s, m, c, k, p]
        wt = wt.transpose(0, 4, 1, 3, 2).reshape(S_PER_CORE, 128, kt * mt * 128)
        wparts.append(_to_fp8(wt * W_SCALE_FP8))
    slab = np.ascontiguousarray(np.concatenate([ximg] + wparts, axis=2))
    assert slab.shape[2] == TOT_COLS

    bias = np.zeros((S_PER_CORE, 128, BIAS_COLS), np.float32)
    for li, (cin, cout) in enumerate(LAYERS):
        b = inputs[f"target_fc{li + 1}b"][sl]  # [S, Cout]
        bias[:, :, BIAS_COL0[li] : BIAS_COL0[li] + cout // 128] = b.reshape(
            S_PER_CORE, cout // 128, 128
        ).transpose(0, 2, 1)
    bias[:, 0, 15] = inputs["target_fc5b"][sl, 0]
    bias = np.ascontiguousarray(bias.transpose(1, 0, 2).reshape(128, -1))

    return {"slab": slab, "w5": w5img, "bias": bias}


def kernel(**inputs):
    inputs = {k: np.asarray(v) for k, v in inputs.items()}
    nc = _get_nc()
    in_maps = [_prep_core(inputs, c) for c in range(N_CORES)]
    res = run_bass_kernel_spmd(nc, in_maps, list(range(N_CORES)))
    out = np.concatenate([np.asarray(res.results[c]["out"]) for c in range(N_CORES)], axis=0)
    return out.reshape(B, 8, 8).astype(np.float32)
